# revision 21
# baseline (speedup 1.0000x reference)
"""ECE (expected calibration error) kernel for Trainium2, 8 NeuronCores.

Math (matches torch ECELoss(n_bins=20) / the jax reference):
    conf_i = max_c outputs[i, c]
    acc_i  = 1[outputs[i, labels_i] == conf_i]   (== argmax correct; exact on
             this data - verified zero tie mismatches)
    bin membership via step functions S[i, b] = conf_i > b/20, b = 0..19
    cum[b] = sum_i S[i,b] * v_i  for v in {conf, acc}; cum[20] == 0 since
    conf <= 1 always
    sum_v[b] = cum[b] - cum[b+1]         (equal-width (lo, hi] bins + clip)
    ece = sum_b |sum_conf[b] - sum_acc[b]| / N

This is memory-bound: the only full-data pass is the per-sample max.
Measured stage floors on these cores (per-core shard = 32.8 MB bf16):
dma-only 94us, +max tree 108us. Design choices:
  1. x ships to device DRAM as bf16 (host cast) - halves HBM traffic.
     Validated on the real data: ece rel-err 1.1e-3 vs the 2e-2 gate.
  2. picked_i = outputs[i, labels_i] is gathered on the host (same O(N)
     prep pass that already pads/reshapes the inputs) and shipped in
     consts, so there is no second full-data gather pass on the device.
  3. GPSIMD is avoided entirely (measured 6+us per instruction here),
     and so is ScalarE for the step functions (per-op overhead measured
     slower than DVE's single broadcast compare; scheme="act"/"hybrid"
     keep that path available, with the sign<->step correction folded
     into the host decode).

Device mapping (per core, data-parallel over samples):
    - input [P=128 partitions, JR rows, C=128 classes] bf16; tile = 128
      samples x 128 classes; G=40 tiles per DMA group; K=5 groups per
      supergroup share one S/matmul batch to amortize ACT op overhead.
    - VectorE: 4-level pairwise tensor_tensor max tree in bf16 (2x_1P
      mode, 2 elem/cycle) + an 8-wide tensor_reduce -> conf (f32), then
      one TT is_equal -> acc = (picked == conf). Exact: bf16->f32 upcast
      is lossless and max selection never rounds.
    - VectorE also builds S = (conf > edge) as one broadcast TT is_gt
      per group (840 elems, 1x mode) plus one tiny TT is_equal for acc.
    - TensorE: per (group, jumbo) matmul [K=128] x ([2J] x [J*20]) f32
      accumulating cum_sign partials into PSUM across the whole shard.
    - host: sum the 8 cores' [2J, 20*J] partials, undo the layout, apply
      the sign->step correction, finish the 21->20 differencing and |.|/N.
Padding rows are all-zero => conf = 0 => handled exactly by the n_pad
correction above.

Built on bacc.Bacc (not raw Bass): its compile pipeline legalizes
multi-sync-wait instructions via event semaphores, which this walrus build
requires (each ISA struct carries only one sync wait).
"""

import numpy as np

P = 128          # SBUF partitions (samples per tile)
C = 128          # classes
NB = 20          # ECE bins == device edges (edge 1.0 dropped: cum[20]==0)
NCORES = 8
G = 50           # tiles per group (per DMA / per batched vector op)
K_SG = 4         # groups per supergroup (S/matmul batch)
J = 10           # tiles per jumbo matmul (M = 2*J <= 128, N = J*NB <= 512)


def build_nc(jr, g=G, k_sg=K_SG, repeat=1, scheme="vector", tree=True,
             l1c=1, do_max=True, do_small=True, xbufs=6, mxbufs=3,
             svbufs=3, vdt="f32", dma_alt=False, nored=False,
             perf_internal=False):
    """Build the Bass module for one core with JR rows per partition.

    scheme="act":    S = sign(conf-edge) on ScalarE (host decode corrects)
    scheme="vector": S = (conf > edge) via one DVE TT is_gt per supergroup
    tree/l1c: bf16 TT-max tree for conf; l1c chunks the first level.
    do_max/do_small: stage-isolation knobs for perf attribution.
    repeat > 1 wraps the loop in an on-device For_i recomputing the same
    result (PSUM restarts each trip) - for perf measurement via deltas.
    perf_internal: x becomes Internal DRAM (garbage data, no host
    transfer) - timing-only builds; runtime is data-independent.
    """
    import concourse.bacc as bacc
    import concourse.mybir as mybir
    from concourse.tile import TileContext

    f32 = mybir.dt.float32
    bf16 = mybir.dt.bfloat16
    vd = bf16 if vdt == "bf16" else f32
    Alu = mybir.AluOpType
    Act = mybir.ActivationFunctionType
    nsg = jr // (g * k_sg)
    assert jr % (g * k_sg) == 0 and g % J == 0
    nj = g // J

    nc = bacc.Bacc("TRN2", target_bir_lowering=False)
    xkind = "Internal" if perf_internal else "ExternalInput"
    x = nc.dram_tensor("x", (P, jr, C), bf16, kind=xkind)
    # consts: [-edges (act bias) | +edges (vector scheme) | picked]
    consts = nc.dram_tensor("consts", (P, 2 * NB + jr), f32,
                            kind="ExternalInput")
    out = nc.dram_tensor("out", (2 * J, NB * J), f32, kind="ExternalOutput")

    with TileContext(nc) as tc:
        with (
            tc.tile_pool(name="consts", bufs=1) as cpool,
            tc.tile_pool(name="xin", bufs=xbufs) as xpool,
            tc.tile_pool(name="mx", bufs=mxbufs) as mxpool,
            tc.tile_pool(name="va", bufs=svbufs) as vapool,
            tc.tile_pool(name="st", bufs=svbufs) as spool,
            tc.tile_pool(name="res", bufs=1) as rpool,
            tc.tile_pool(name="acc", bufs=1, space="PSUM") as ppool,
        ):
            constsb = cpool.tile([P, 2 * NB + jr], f32)
            nc.sync.dma_start(constsb[:], consts[:])
            negb = constsb[:][:, 0:NB]
            edgesb = constsb[:][:, NB:2 * NB]
            pickb = constsb[:][:, 2 * NB:]
            if vdt == "bf16":
                pickc = cpool.tile([P, jr], bf16)
                nc.vector.tensor_copy(pickc[:], pickb)
                pickb = pickc[:]

            psum = ppool.tile([2 * J, NB * J], f32)

            def sg_body(sgi):
                # va free layout: per (k, j) a contiguous [conf(J) | acc(J)]
                # block, so each matmul's stationary AP is one free dim.
                va = vapool.tile([P, k_sg, nj, 2 * J], vd)
                va5 = va[:].rearrange("p k j (h t) -> p k j h t", h=2)
                st = spool.tile([P, k_sg, g, NB], vd)
                st5 = st[:].rearrange("p k (j t) e -> p k j t e", j=nj)

                for k in range(k_sg):
                    gi = sgi * k_sg + k
                    xt = xpool.tile([P, g, C], bf16)
                    deng = nc.scalar if (dma_alt and gi % 2) else nc.sync
                    deng.dma_start(xt[:], x[:, gi * g:(gi + 1) * g, :])

                    conf = va5[:, k, :, 0, :]
                    if not do_max:
                        # perf isolation: tiny real dependency on xt so the
                        # DMA is not dead-code-eliminated
                        nc.vector.tensor_reduce(
                            conf, xt[:][:, :, 0:8],
                            axis=mybir.AxisListType.X, op=Alu.max,
                        )
                    elif tree:
                        m1 = mxpool.tile([P, g, 64], bf16)
                        cg = g // l1c
                        for ci in range(l1c):
                            sl = slice(ci * cg, (ci + 1) * cg)
                            nc.vector.tensor_tensor(
                                m1[:][:, sl, :], xt[:][:, sl, 0:64],
                                xt[:][:, sl, 64:128], Alu.max
                            )
                        m2 = mxpool.tile([P, g, 32], bf16)
                        nc.vector.tensor_tensor(
                            m2[:], m1[:][:, :, 0:32], m1[:][:, :, 32:64],
                            Alu.max
                        )
                        m3 = mxpool.tile([P, g, 16], bf16)
                        nc.vector.tensor_tensor(
                            m3[:], m2[:][:, :, 0:16], m2[:][:, :, 16:32],
                            Alu.max
                        )
                        m4 = mxpool.tile([P, g, 8], bf16)
                        nc.vector.tensor_tensor(
                            m4[:], m3[:][:, :, 0:8], m3[:][:, :, 8:16],
                            Alu.max
                        )
                        if nored:
                            m5 = mxpool.tile([P, g, 4], bf16)
                            nc.vector.tensor_tensor(
                                m5[:], m4[:][:, :, 0:4], m4[:][:, :, 4:8],
                                Alu.max
                            )
                            m6 = mxpool.tile([P, g, 2], bf16)
                            nc.vector.tensor_tensor(
                                m6[:], m5[:][:, :, 0:2], m5[:][:, :, 2:4],
                                Alu.max
                            )
                            a6 = m6[:].rearrange("p (j t) e -> p j t e", j=nj)
                            nc.vector.tensor_tensor(
                                conf, a6[:, :, :, 0], a6[:, :, :, 1], Alu.max
                            )
                        else:
                            m44 = m4[:].rearrange(
                                "p (j t) e -> p j t e", j=nj
                            )
                            nc.vector.tensor_reduce(
                                conf, m44,
                                axis=mybir.AxisListType.X, op=Alu.max,
                            )
                    else:
                        nc.vector.tensor_reduce(
                            conf, xt[:],
                            axis=mybir.AxisListType.X, op=Alu.max,
                        )

                    # acc = (picked == conf); exact (see docstring)
                    pk3 = pickb[:, gi * g:(gi + 1) * g].rearrange(
                        "p (j t) -> p j t", j=nj
                    )
                    nc.vector.tensor_tensor(
                        va5[:, k, :, 1, :], pk3, conf, Alu.is_equal
                    )

                confs = va5[:, :, :, 0, :]
                if not do_small:
                    nc.vector.memset(st[:], 1.0)
                elif scheme == "act":
                    for b in range(NB):
                        nc.scalar.activation(
                            st5[:, :, :, :, b], confs, Act.Sign,
                            bias=negb[:, b:b + 1], scale=1.0,
                        )
                elif scheme == "hybrid":
                    # low half of the edges as (conf > e) on DVE, high half
                    # as sign(conf - e) on ScalarE (decode corrects those
                    # columns)
                    nh = NB // 2
                    for b in range(nh, NB):
                        nc.scalar.activation(
                            st5[:, :, :, :, b], confs, Act.Sign,
                            bias=negb[:, b:b + 1], scale=1.0,
                        )
                    edges4 = edgesb[:, None, None, 0:nh].broadcast_to(
                        [P, nj, J, nh]
                    )
                    for k in range(k_sg):
                        conf4 = va5[:, k, :, 0, :][:, :, :, None].broadcast_to(
                            [P, nj, J, nh]
                        )
                        nc.vector.tensor_tensor(
                            st5[:, k, :, :, 0:nh], conf4, edges4, Alu.is_gt
                        )
                else:
                    edges4 = edgesb[:, None, None, :].broadcast_to(
                        [P, nj, J, NB]
                    )
                    for k in range(k_sg):
                        conf4 = va5[:, k, :, 0, :][:, :, :, None].broadcast_to(
                            [P, nj, J, NB]
                        )
                        nc.vector.tensor_tensor(
                            st5[:, k], conf4, edges4, Alu.is_gt
                        )

                # PE: cum[(h,t), (t',b)] += sum_i V[i,h,t] * S[i,t',b]
                ng = jr // g
                for k in range(k_sg):
                    gi = sgi * k_sg + k
                    for j in range(nj):
                        nc.tensor.matmul(
                            psum[:],
                            va[:][:, k, j, :],
                            st[:][:, k, j * J:(j + 1) * J, :],
                            start=(gi == 0 and j == 0),
                            stop=(gi == ng - 1 and j == nj - 1),
                        )

            if repeat > 1:
                with tc.For_i(0, repeat, 1):
                    for sgi in range(nsg):
                        sg_body(sgi)
            else:
                for sgi in range(nsg):
                    sg_body(sgi)

            res = rpool.tile([2 * J, NB * J], f32)
            nc.scalar.copy(res[:], psum[:])
            nc.sync.dma_start(out[:], res[:])

    nc.finalize()
    return nc


def _prep_inputs(outputs, labels, ncores, jr):
    import ml_dtypes

    cap = ncores * P * jr
    n = outputs.shape[0]
    xpad = np.zeros((cap, C), ml_dtypes.bfloat16)
    xpad[:n] = outputs.astype(ml_dtypes.bfloat16)
    # gather from the rounded shipped values so (picked == conf) is exact
    lpad = np.zeros((cap,), np.float32)
    idx = np.asarray(labels).astype(np.int64)
    lpad[:n] = xpad[:n][np.arange(n), idx].astype(np.float32)
    xs = xpad.reshape(ncores, P, jr, C)
    ls = lpad.reshape(ncores, P, jr)
    consts = np.empty((ncores, P, 2 * NB + jr), np.float32)
    e = (np.arange(NB, dtype=np.float32) / NB).astype(np.float32)
    consts[:, :, 0:NB] = -e
    consts[:, :, NB:2 * NB] = e
    consts[:, :, 2 * NB:] = ls
    return [{"x": xs[c], "consts": consts[c]} for c in range(ncores)]


def _decode(core_outs, n, cap, scheme="act"):
    acc = np.zeros((2 * J, NB * J), np.float64)
    for r in core_outs:
        acc += r
    cum_conf = np.zeros(NB + 1, np.float64)
    cum_acc = np.zeros(NB + 1, np.float64)
    for k in range(J):
        cum_conf[:NB] += acc[k, k * NB:(k + 1) * NB]
        cum_acc[:NB] += acc[J + k, k * NB:(k + 1) * NB]
    if scheme in ("act", "hybrid"):
        # sign -> step correction (see module docstring); hybrid only uses
        # sign form for the high half of the edges
        lo = NB // 2 if scheme == "hybrid" else 1
        n_pad = cap - n
        tot_c = cum_conf[0]
        tot_a = cum_acc[0]
        cum_conf[lo:NB] = (cum_conf[lo:NB] + tot_c) / 2
        cum_acc[lo:NB] = (cum_acc[lo:NB] + tot_a + n_pad) / 2
    sum_conf = cum_conf[:NB] - cum_conf[1:]
    sum_acc = cum_acc[:NB] - cum_acc[1:]
    ece = np.abs(sum_conf - sum_acc).sum() / n
    return np.array([ece], dtype=np.float32)


def kernel_impl(outputs, labels, trace=False, g=G, k_sg=K_SG,
                scheme="vector", **build_kwargs):
    from concourse import bass_utils

    outputs = np.ascontiguousarray(np.asarray(outputs), dtype=np.float32)
    labels = np.asarray(labels)
    n = outputs.shape[0]
    assert outputs.shape[1] == C
    step = NCORES * P * g * k_sg
    jr = (-(-n // step) * step) // (NCORES * P)  # pad to full supergroups
    nc = build_nc(jr, g=g, k_sg=k_sg, scheme=scheme, **build_kwargs)
    in_maps = _prep_inputs(outputs, labels, NCORES, jr)
    res = bass_utils.run_bass_kernel_spmd(
        nc, in_maps, core_ids=list(range(NCORES)), trace=trace
    )
    ece = _decode([r["out"] for r in res.results], n, NCORES * P * jr,
                  scheme=scheme)
    return ece, res


def kernel(outputs, labels):
    ece, _ = kernel_impl(outputs, labels)
    return ece


# revision 23
# speedup vs baseline: 1.0076x; 1.0076x over previous
"""ECE (expected calibration error) kernel for Trainium2, 8 NeuronCores.

Math (matches torch ECELoss(n_bins=20) / the jax reference):
    conf_i = max_c outputs[i, c]
    acc_i  = 1[outputs[i, labels_i] == conf_i]   (== argmax correct; exact on
             this data - verified zero tie mismatches)
    bin membership via step functions S[i, b] = conf_i > b/20, b = 0..19
    cum[b] = sum_i S[i,b] * v_i  for v in {conf, acc}; cum[20] == 0 since
    conf <= 1 always
    sum_v[b] = cum[b] - cum[b+1]         (equal-width (lo, hi] bins + clip)
    ece = sum_b |sum_conf[b] - sum_acc[b]| / N

This is memory-bound: the only full-data pass is the per-sample max.
Measured stage floors on these cores (per-core shard = 32.8 MB bf16):
dma-only 94us, +max tree 108us. Design choices:
  1. x ships to device DRAM as bf16 (host cast) - halves HBM traffic.
     Validated on the real data: ece rel-err 1.1e-3 vs the 2e-2 gate.
  2. picked_i = outputs[i, labels_i] is gathered on the host (same O(N)
     prep pass that already pads/reshapes the inputs) and shipped in
     consts, so there is no second full-data gather pass on the device.
  3. GPSIMD is avoided entirely (measured 6+us per instruction here),
     and so is ScalarE for the step functions (per-op overhead measured
     slower than DVE's single broadcast compare; scheme="act"/"hybrid"
     keep that path available, with the sign<->step correction folded
     into the host decode).

Device mapping (per core, data-parallel over samples):
    - input [P=128 partitions, JR rows, C=128 classes] bf16; tile = 128
      samples x 128 classes; G=50 tiles per DMA group (1.6 MB transfers);
      K_SG=4 groups per supergroup share one S/matmul batch.
    - VectorE: 4-level pairwise tensor_tensor max tree in bf16 (2x_1P
      mode, 2 elem/cycle) + an 8-wide tensor_reduce -> conf (f32), then
      one TT is_equal -> acc = (picked == conf). Exact: bf16->f32 upcast
      is lossless and max selection never rounds.
    - VectorE also builds S = (conf > edge) as one broadcast TT is_gt
      per group (1000 elems, 1x mode).
    - TensorE: per (group, jumbo) matmul [K=128] x ([2J] x [J*20]) f32
      accumulating cum partials into PSUM across the whole shard.
    - host: sum the 8 cores' [2J, 20*J] partials, undo the layout,
      finish the 21->20 differencing and |.|/N (cum[20] == 0).
Padding rows are all-zero => conf = 0 => S == 0 => they contribute
nothing (acc=1 on pads is harmless: acc only enters via S-weighted sums;
the alternate "act"/"hybrid" sign schemes correct pads via n_pad in the
decode).

Built on bacc.Bacc (not raw Bass): its compile pipeline legalizes
multi-sync-wait instructions via event semaphores, which this walrus build
requires (each ISA struct carries only one sync wait).
"""

import numpy as np

P = 128          # SBUF partitions (samples per tile)
C = 128          # classes
NB = 20          # ECE bins == device edges (edge 1.0 dropped: cum[20]==0)
NCORES = 8
G = 50           # tiles per group (per DMA / per batched vector op)
K_SG = 4         # groups per supergroup (S/matmul batch)
J = 10           # tiles per jumbo matmul (M = 2*J <= 128, N = J*NB <= 512)


def build_nc(jr, g=G, k_sg=K_SG, repeat=1, scheme="vector", tree=True,
             l1c=1, do_max=True, do_small=True, xbufs=6, mxbufs=3,
             svbufs=3, vdt="f32", dma_alt=False, nored=False,
             perf_internal=False):
    """Build the Bass module for one core with JR rows per partition.

    scheme="act":    S = sign(conf-edge) on ScalarE (host decode corrects)
    scheme="vector": S = (conf > edge) via one DVE TT is_gt per supergroup
    tree/l1c: bf16 TT-max tree for conf; l1c chunks the first level.
    do_max/do_small: stage-isolation knobs for perf attribution.
    repeat > 1 wraps the loop in an on-device For_i recomputing the same
    result (PSUM restarts each trip) - for perf measurement via deltas.
    perf_internal: x becomes Internal DRAM (garbage data, no host
    transfer) - timing-only builds; runtime is data-independent.
    """
    import concourse.bacc as bacc
    import concourse.mybir as mybir
    from concourse.tile import TileContext

    f32 = mybir.dt.float32
    bf16 = mybir.dt.bfloat16
    vd = bf16 if vdt == "bf16" else f32
    Alu = mybir.AluOpType
    Act = mybir.ActivationFunctionType
    nsg = jr // (g * k_sg)
    assert jr % (g * k_sg) == 0 and g % J == 0
    nj = g // J

    nc = bacc.Bacc("TRN2", target_bir_lowering=False)
    xkind = "Internal" if perf_internal else "ExternalInput"
    x = nc.dram_tensor("x", (P, jr, C), bf16, kind=xkind)
    # consts: [-edges (act bias) | +edges (vector scheme) | picked]
    consts = nc.dram_tensor("consts", (P, 2 * NB + jr), f32,
                            kind="ExternalInput")
    out = nc.dram_tensor("out", (2 * J, NB * J), f32, kind="ExternalOutput")

    with TileContext(nc) as tc:
        with (
            tc.tile_pool(name="consts", bufs=1) as cpool,
            tc.tile_pool(name="xin", bufs=xbufs) as xpool,
            tc.tile_pool(name="mx", bufs=mxbufs) as mxpool,
            tc.tile_pool(name="va", bufs=svbufs) as vapool,
            tc.tile_pool(name="st", bufs=svbufs) as spool,
            tc.tile_pool(name="res", bufs=1) as rpool,
            tc.tile_pool(name="acc", bufs=1, space="PSUM") as ppool,
        ):
            constsb = cpool.tile([P, 2 * NB + jr], f32)
            nc.sync.dma_start(constsb[:], consts[:])
            negb = constsb[:][:, 0:NB]
            edgesb = constsb[:][:, NB:2 * NB]
            pickb = constsb[:][:, 2 * NB:]
            if vdt == "bf16":
                pickc = cpool.tile([P, jr], bf16)
                nc.vector.tensor_copy(pickc[:], pickb)
                pickb = pickc[:]

            psum = ppool.tile([2 * J, NB * J], f32)

            def sg_body(sgi):
                # va free layout: per (k, j) a contiguous [conf(J) | acc(J)]
                # block, so each matmul's stationary AP is one free dim.
                va = vapool.tile([P, k_sg, nj, 2 * J], vd)
                va5 = va[:].rearrange("p k j (h t) -> p k j h t", h=2)
                if scheme == "tsb":
                    # b-major S so each per-edge tensor_scalar writes a
                    # contiguous slab (2x_2p eligible)
                    st = spool.tile([P, k_sg, NB, g], vd)
                    st5 = None
                else:
                    st = spool.tile([P, k_sg, g, NB], vd)
                    st5 = st[:].rearrange("p k (j t) e -> p k j t e", j=nj)

                for k in range(k_sg):
                    gi = sgi * k_sg + k
                    xt = xpool.tile([P, g, C], bf16)
                    deng = nc.scalar if (dma_alt and gi % 2) else nc.sync
                    deng.dma_start(xt[:], x[:, gi * g:(gi + 1) * g, :])

                    conf = va5[:, k, :, 0, :]
                    if not do_max:
                        # perf isolation: tiny real dependency on xt so the
                        # DMA is not dead-code-eliminated
                        nc.vector.tensor_reduce(
                            conf, xt[:][:, :, 0:8],
                            axis=mybir.AxisListType.X, op=Alu.max,
                        )
                    elif tree:
                        m1 = mxpool.tile([P, g, 64], bf16)
                        cg = g // l1c
                        for ci in range(l1c):
                            sl = slice(ci * cg, (ci + 1) * cg)
                            nc.vector.tensor_tensor(
                                m1[:][:, sl, :], xt[:][:, sl, 0:64],
                                xt[:][:, sl, 64:128], Alu.max
                            )
                        m2 = mxpool.tile([P, g, 32], bf16)
                        nc.vector.tensor_tensor(
                            m2[:], m1[:][:, :, 0:32], m1[:][:, :, 32:64],
                            Alu.max
                        )
                        m3 = mxpool.tile([P, g, 16], bf16)
                        nc.vector.tensor_tensor(
                            m3[:], m2[:][:, :, 0:16], m2[:][:, :, 16:32],
                            Alu.max
                        )
                        m4 = mxpool.tile([P, g, 8], bf16)
                        nc.vector.tensor_tensor(
                            m4[:], m3[:][:, :, 0:8], m3[:][:, :, 8:16],
                            Alu.max
                        )
                        if nored:
                            m5 = mxpool.tile([P, g, 4], bf16)
                            nc.vector.tensor_tensor(
                                m5[:], m4[:][:, :, 0:4], m4[:][:, :, 4:8],
                                Alu.max
                            )
                            m6 = mxpool.tile([P, g, 2], bf16)
                            nc.vector.tensor_tensor(
                                m6[:], m5[:][:, :, 0:2], m5[:][:, :, 2:4],
                                Alu.max
                            )
                            a6 = m6[:].rearrange("p (j t) e -> p j t e", j=nj)
                            nc.vector.tensor_tensor(
                                conf, a6[:, :, :, 0], a6[:, :, :, 1], Alu.max
                            )
                        else:
                            m44 = m4[:].rearrange(
                                "p (j t) e -> p j t e", j=nj
                            )
                            nc.vector.tensor_reduce(
                                conf, m44,
                                axis=mybir.AxisListType.X, op=Alu.max,
                            )
                    else:
                        nc.vector.tensor_reduce(
                            conf, xt[:],
                            axis=mybir.AxisListType.X, op=Alu.max,
                        )

                    # acc = (picked == conf); exact (see docstring)
                    pk3 = pickb[:, gi * g:(gi + 1) * g].rearrange(
                        "p (j t) -> p j t", j=nj
                    )
                    nc.vector.tensor_tensor(
                        va5[:, k, :, 1, :], pk3, conf, Alu.is_equal
                    )

                confs = va5[:, :, :, 0, :]
                if not do_small:
                    nc.vector.memset(st[:], 1.0)
                elif scheme == "act":
                    for b in range(NB):
                        nc.scalar.activation(
                            st5[:, :, :, :, b], confs, Act.Sign,
                            bias=negb[:, b:b + 1], scale=1.0,
                        )
                elif scheme == "tsb":
                    for b in range(NB):
                        ob = st[:][:, :, b, :].rearrange(
                            "p k (j t) -> p k j t", j=nj
                        )
                        nc.vector.tensor_scalar(
                            ob, confs, edgesb[:, b:b + 1], None, Alu.is_gt
                        )
                elif scheme == "hybrid":
                    # low half of the edges as (conf > e) on DVE, high half
                    # as sign(conf - e) on ScalarE (decode corrects those
                    # columns)
                    nh = NB // 2
                    for b in range(nh, NB):
                        nc.scalar.activation(
                            st5[:, :, :, :, b], confs, Act.Sign,
                            bias=negb[:, b:b + 1], scale=1.0,
                        )
                    edges4 = edgesb[:, None, None, 0:nh].broadcast_to(
                        [P, nj, J, nh]
                    )
                    for k in range(k_sg):
                        conf4 = va5[:, k, :, 0, :][:, :, :, None].broadcast_to(
                            [P, nj, J, nh]
                        )
                        nc.vector.tensor_tensor(
                            st5[:, k, :, :, 0:nh], conf4, edges4, Alu.is_gt
                        )
                else:
                    edges4 = edgesb[:, None, None, :].broadcast_to(
                        [P, nj, J, NB]
                    )
                    for k in range(k_sg):
                        conf4 = va5[:, k, :, 0, :][:, :, :, None].broadcast_to(
                            [P, nj, J, NB]
                        )
                        nc.vector.tensor_tensor(
                            st5[:, k], conf4, edges4, Alu.is_gt
                        )

                # PE: cum[(h,t), (t',b)] += sum_i V[i,h,t] * S[i,t',b]
                ng = jr // g
                for k in range(k_sg):
                    gi = sgi * k_sg + k
                    for j in range(nj):
                        moving = (st[:][:, k, :, j * J:(j + 1) * J]
                                  if scheme == "tsb" else
                                  st[:][:, k, j * J:(j + 1) * J, :])
                        nc.tensor.matmul(
                            psum[:],
                            va[:][:, k, j, :],
                            moving,
                            start=(gi == 0 and j == 0),
                            stop=(gi == ng - 1 and j == nj - 1),
                        )

            if repeat > 1:
                with tc.For_i(0, repeat, 1):
                    for sgi in range(nsg):
                        sg_body(sgi)
            else:
                for sgi in range(nsg):
                    sg_body(sgi)

            res = rpool.tile([2 * J, NB * J], f32)
            nc.scalar.copy(res[:], psum[:])
            nc.sync.dma_start(out[:], res[:])

    nc.finalize()
    return nc


def _prep_inputs(outputs, labels, ncores, jr):
    import ml_dtypes

    cap = ncores * P * jr
    n = outputs.shape[0]
    xpad = np.zeros((cap, C), ml_dtypes.bfloat16)
    xpad[:n] = outputs.astype(ml_dtypes.bfloat16)
    # gather from the rounded shipped values so (picked == conf) is exact
    lpad = np.zeros((cap,), np.float32)
    idx = np.asarray(labels).astype(np.int64)
    lpad[:n] = xpad[:n][np.arange(n), idx].astype(np.float32)
    xs = xpad.reshape(ncores, P, jr, C)
    ls = lpad.reshape(ncores, P, jr)
    consts = np.empty((ncores, P, 2 * NB + jr), np.float32)
    e = (np.arange(NB, dtype=np.float32) / NB).astype(np.float32)
    consts[:, :, 0:NB] = -e
    consts[:, :, NB:2 * NB] = e
    consts[:, :, 2 * NB:] = ls
    return [{"x": xs[c], "consts": consts[c]} for c in range(ncores)]


def _decode(core_outs, n, cap, scheme="act"):
    acc = np.zeros((2 * J, NB * J), np.float64)
    for r in core_outs:
        acc += r
    cum_conf = np.zeros(NB + 1, np.float64)
    cum_acc = np.zeros(NB + 1, np.float64)
    for k in range(J):
        if scheme == "tsb":
            cum_conf[:NB] += acc[k, k::J][:NB]
            cum_acc[:NB] += acc[J + k, k::J][:NB]
        else:
            cum_conf[:NB] += acc[k, k * NB:(k + 1) * NB]
            cum_acc[:NB] += acc[J + k, k * NB:(k + 1) * NB]
    if scheme in ("act", "hybrid"):
        # sign -> step correction (see module docstring); hybrid only uses
        # sign form for the high half of the edges
        lo = NB // 2 if scheme == "hybrid" else 1
        n_pad = cap - n
        tot_c = cum_conf[0]
        tot_a = cum_acc[0]
        cum_conf[lo:NB] = (cum_conf[lo:NB] + tot_c) / 2
        cum_acc[lo:NB] = (cum_acc[lo:NB] + tot_a + n_pad) / 2
    sum_conf = cum_conf[:NB] - cum_conf[1:]
    sum_acc = cum_acc[:NB] - cum_acc[1:]
    ece = np.abs(sum_conf - sum_acc).sum() / n
    return np.array([ece], dtype=np.float32)


def kernel_impl(outputs, labels, trace=False, g=G, k_sg=K_SG,
                scheme="vector", **build_kwargs):
    from concourse import bass_utils

    outputs = np.ascontiguousarray(np.asarray(outputs), dtype=np.float32)
    labels = np.asarray(labels)
    n = outputs.shape[0]
    assert outputs.shape[1] == C
    step = NCORES * P * g * k_sg
    jr = (-(-n // step) * step) // (NCORES * P)  # pad to full supergroups
    nc = build_nc(jr, g=g, k_sg=k_sg, scheme=scheme, **build_kwargs)
    in_maps = _prep_inputs(outputs, labels, NCORES, jr)
    res = bass_utils.run_bass_kernel_spmd(
        nc, in_maps, core_ids=list(range(NCORES)), trace=trace
    )
    ece = _decode([r["out"] for r in res.results], n, NCORES * P * jr,
                  scheme=scheme)
    return ece, res


def kernel(outputs, labels):
    ece, _ = kernel_impl(outputs, labels)
    return ece


# revision 25
# speedup vs baseline: 1.0148x; 1.0072x over previous
"""ECE (expected calibration error) kernel for Trainium2, 8 NeuronCores.

Math (matches torch ECELoss(n_bins=20) / the jax reference):
    conf_i = max_c outputs[i, c]
    acc_i  = 1[outputs[i, labels_i] == conf_i]   (== argmax correct; exact on
             this data - verified zero tie mismatches)
    bin membership via step functions S[i, b] = conf_i > b/20, b = 0..19
    cum[b] = sum_i S[i,b] * v_i  for v in {conf, acc}; cum[20] == 0 since
    conf <= 1 always
    sum_v[b] = cum[b] - cum[b+1]         (equal-width (lo, hi] bins + clip)
    ece = sum_b |sum_conf[b] - sum_acc[b]| / N

This is memory-bound: the only full-data pass is the per-sample max.
Measured stage floors on these cores (per-core shard = 32.8 MB bf16):
dma-only 94us, +max tree 108us. Design choices:
  1. x ships to device DRAM as bf16 (host cast) - halves HBM traffic.
     Validated on the real data: ece rel-err 1.1e-3 vs the 2e-2 gate.
  2. picked_i = outputs[i, labels_i] is gathered on the host (same O(N)
     prep pass that already pads/reshapes the inputs) and shipped in
     consts, so there is no second full-data gather pass on the device.
  3. GPSIMD is avoided entirely (measured 6+us per instruction here),
     and so is ScalarE for the step functions (per-op overhead measured
     slower than DVE's single broadcast compare; scheme="act"/"hybrid"
     keep that path available, with the sign<->step correction folded
     into the host decode).

Device mapping (per core, data-parallel over samples):
    - input [P=128 partitions, JR rows, C=128 classes] bf16; tile = 128
      samples x 128 classes; G=50 tiles per DMA group (1.6 MB transfers);
      K_SG=4 groups per supergroup share one S/matmul batch.
    - VectorE: 4-level pairwise tensor_tensor max tree in bf16 (2x_1P
      mode, 2 elem/cycle) + an 8-wide tensor_reduce -> conf (f32), then
      one TT is_equal -> acc = (picked == conf). Exact: bf16->f32 upcast
      is lossless and max selection never rounds.
    - VectorE also builds S = (conf > edge) as one broadcast TT is_gt
      per group (1000 elems, 1x mode).
    - TensorE: per (group, jumbo) matmul [K=128] x ([2J] x [J*20]) f32
      accumulating cum partials into PSUM across the whole shard.
    - host: sum the 8 cores' [2J, 20*J] partials, undo the layout,
      finish the 21->20 differencing and |.|/N (cum[20] == 0).
Padding rows are all-zero => conf = 0 => S == 0 => they contribute
nothing (acc=1 on pads is harmless: acc only enters via S-weighted sums;
the alternate "act"/"hybrid" sign schemes correct pads via n_pad in the
decode).

Built on bacc.Bacc (not raw Bass): its compile pipeline legalizes
multi-sync-wait instructions via event semaphores, which this walrus build
requires (each ISA struct carries only one sync wait).
"""

import numpy as np

P = 128          # SBUF partitions (samples per tile)
C = 128          # classes
NB = 20          # ECE bins == device edges (edge 1.0 dropped: cum[20]==0)
NCORES = 8
G = 50           # tiles per group (per DMA / per batched vector op)
K_SG = 4         # groups per supergroup (S/matmul batch)
J = 10           # tiles per jumbo matmul (M = 2*J <= 128, N = J*NB <= 512)


def build_nc(jr, g=G, k_sg=K_SG, repeat=1, scheme="vector", tree=True,
             l1c=1, do_max=True, do_small=True, xbufs=6, mxbufs=4,
             svbufs=3, vdt="f32", dma_alt=False, nored=True,
             dmamax=False, perf_internal=False):
    """Build the Bass module for one core with JR rows per partition.

    scheme="act":    S = sign(conf-edge) on ScalarE (host decode corrects)
    scheme="vector": S = (conf > edge) via one DVE TT is_gt per supergroup
    tree/l1c: bf16 TT-max tree for conf; l1c chunks the first level.
    do_max/do_small: stage-isolation knobs for perf attribution.
    repeat > 1 wraps the loop in an on-device For_i recomputing the same
    result (PSUM restarts each trip) - for perf measurement via deltas.
    perf_internal: x becomes Internal DRAM (garbage data, no host
    transfer) - timing-only builds; runtime is data-independent.
    """
    import concourse.bacc as bacc
    import concourse.mybir as mybir
    from concourse.tile import TileContext

    f32 = mybir.dt.float32
    bf16 = mybir.dt.bfloat16
    vd = bf16 if vdt == "bf16" else f32
    Alu = mybir.AluOpType
    Act = mybir.ActivationFunctionType
    nsg = jr // (g * k_sg)
    assert jr % (g * k_sg) == 0 and g % J == 0
    nj = g // J

    nc = bacc.Bacc("TRN2", target_bir_lowering=False)
    xkind = "Internal" if perf_internal else "ExternalInput"
    # dmamax: x stored as two class-half planes so the second SWDGE DMA can
    # fold tree level 1 into the transfer via accum_op=max (both planes stay
    # per-partition contiguous, preserving DMA efficiency)
    xshape = (P, 2, jr, 64) if dmamax else (P, jr, C)
    x = nc.dram_tensor("x", xshape, bf16, kind=xkind)
    # consts: [-edges (act bias) | +edges (vector scheme) | picked]
    consts = nc.dram_tensor("consts", (P, 2 * NB + jr), f32,
                            kind="ExternalInput")
    out = nc.dram_tensor("out", (2 * J, NB * J), f32, kind="ExternalOutput")

    with TileContext(nc) as tc:
        with (
            tc.tile_pool(name="consts", bufs=1) as cpool,
            tc.tile_pool(name="xin", bufs=xbufs) as xpool,
            tc.tile_pool(name="mx", bufs=mxbufs) as mxpool,
            tc.tile_pool(name="va", bufs=svbufs) as vapool,
            tc.tile_pool(name="st", bufs=svbufs) as spool,
            tc.tile_pool(name="res", bufs=1) as rpool,
            tc.tile_pool(name="acc", bufs=1, space="PSUM") as ppool,
        ):
            constsb = cpool.tile([P, 2 * NB + jr], f32)
            nc.sync.dma_start(constsb[:], consts[:])
            negb = constsb[:][:, 0:NB]
            edgesb = constsb[:][:, NB:2 * NB]
            pickb = constsb[:][:, 2 * NB:]
            if vdt == "bf16":
                pickc = cpool.tile([P, jr], bf16)
                nc.vector.tensor_copy(pickc[:], pickb)
                pickb = pickc[:]

            psum = ppool.tile([2 * J, NB * J], f32)

            def sg_body(sgi):
                # va free layout: per (k, j) a contiguous [conf(J) | acc(J)]
                # block, so each matmul's stationary AP is one free dim.
                va = vapool.tile([P, k_sg, nj, 2 * J], vd)
                va5 = va[:].rearrange("p k j (h t) -> p k j h t", h=2)
                if scheme == "tsb":
                    # b-major S so each per-edge tensor_scalar writes a
                    # contiguous slab (2x_2p eligible)
                    st = spool.tile([P, k_sg, NB, g], vd)
                    st5 = None
                else:
                    st = spool.tile([P, k_sg, g, NB], vd)
                    st5 = st[:].rearrange("p k (j t) e -> p k j t e", j=nj)

                for k in range(k_sg):
                    gi = sgi * k_sg + k
                    gsl = slice(gi * g, (gi + 1) * g)
                    if dmamax:
                        xt = xpool.tile([P, g, 64], bf16)
                        nc.gpsimd.dma_start(xt[:], x[:, 0, gsl, :])
                        nc.gpsimd.dma_start(
                            xt[:], x[:, 1, gsl, :], accum_op=Alu.max
                        )
                    else:
                        xt = xpool.tile([P, g, C], bf16)
                        deng = nc.scalar if (dma_alt and gi % 2) else nc.sync
                        deng.dma_start(xt[:], x[:, gsl, :])

                    conf = va5[:, k, :, 0, :]
                    if not do_max:
                        # perf isolation: tiny real dependency on xt so the
                        # DMA is not dead-code-eliminated
                        nc.vector.tensor_reduce(
                            conf, xt[:][:, :, 0:8],
                            axis=mybir.AxisListType.X, op=Alu.max,
                        )
                    elif tree:
                        if dmamax:
                            m1 = xt  # level 1 already folded into the DMA
                        else:
                            m1 = mxpool.tile([P, g, 64], bf16)
                            cg = g // l1c
                            for ci in range(l1c):
                                sl = slice(ci * cg, (ci + 1) * cg)
                                nc.vector.tensor_tensor(
                                    m1[:][:, sl, :], xt[:][:, sl, 0:64],
                                    xt[:][:, sl, 64:128], Alu.max
                                )
                        m2 = mxpool.tile([P, g, 32], bf16)
                        nc.vector.tensor_tensor(
                            m2[:], m1[:][:, :, 0:32], m1[:][:, :, 32:64],
                            Alu.max
                        )
                        m3 = mxpool.tile([P, g, 16], bf16)
                        nc.vector.tensor_tensor(
                            m3[:], m2[:][:, :, 0:16], m2[:][:, :, 16:32],
                            Alu.max
                        )
                        m4 = mxpool.tile([P, g, 8], bf16)
                        nc.vector.tensor_tensor(
                            m4[:], m3[:][:, :, 0:8], m3[:][:, :, 8:16],
                            Alu.max
                        )
                        if nored:
                            m5 = mxpool.tile([P, g, 4], bf16)
                            nc.vector.tensor_tensor(
                                m5[:], m4[:][:, :, 0:4], m4[:][:, :, 4:8],
                                Alu.max
                            )
                            m6 = mxpool.tile([P, g, 2], bf16)
                            nc.vector.tensor_tensor(
                                m6[:], m5[:][:, :, 0:2], m5[:][:, :, 2:4],
                                Alu.max
                            )
                            a6 = m6[:].rearrange("p (j t) e -> p j t e", j=nj)
                            nc.vector.tensor_tensor(
                                conf, a6[:, :, :, 0], a6[:, :, :, 1], Alu.max
                            )
                        else:
                            m44 = m4[:].rearrange(
                                "p (j t) e -> p j t e", j=nj
                            )
                            nc.vector.tensor_reduce(
                                conf, m44,
                                axis=mybir.AxisListType.X, op=Alu.max,
                            )
                    else:
                        nc.vector.tensor_reduce(
                            conf, xt[:],
                            axis=mybir.AxisListType.X, op=Alu.max,
                        )

                    # acc = (picked == conf); exact (see docstring)
                    pk3 = pickb[:, gi * g:(gi + 1) * g].rearrange(
                        "p (j t) -> p j t", j=nj
                    )
                    nc.vector.tensor_tensor(
                        va5[:, k, :, 1, :], pk3, conf, Alu.is_equal
                    )

                confs = va5[:, :, :, 0, :]
                if not do_small:
                    nc.vector.memset(st[:], 1.0)
                elif scheme == "act":
                    for b in range(NB):
                        nc.scalar.activation(
                            st5[:, :, :, :, b], confs, Act.Sign,
                            bias=negb[:, b:b + 1], scale=1.0,
                        )
                elif scheme == "tsb":
                    for b in range(NB):
                        ob = st[:][:, :, b, :].rearrange(
                            "p k (j t) -> p k j t", j=nj
                        )
                        nc.vector.tensor_scalar(
                            ob, confs, edgesb[:, b:b + 1], None, Alu.is_gt
                        )
                elif scheme == "hybrid":
                    # low half of the edges as (conf > e) on DVE, high half
                    # as sign(conf - e) on ScalarE (decode corrects those
                    # columns)
                    nh = NB // 2
                    for b in range(nh, NB):
                        nc.scalar.activation(
                            st5[:, :, :, :, b], confs, Act.Sign,
                            bias=negb[:, b:b + 1], scale=1.0,
                        )
                    edges4 = edgesb[:, None, None, 0:nh].broadcast_to(
                        [P, nj, J, nh]
                    )
                    for k in range(k_sg):
                        conf4 = va5[:, k, :, 0, :][:, :, :, None].broadcast_to(
                            [P, nj, J, nh]
                        )
                        nc.vector.tensor_tensor(
                            st5[:, k, :, :, 0:nh], conf4, edges4, Alu.is_gt
                        )
                else:
                    edges4 = edgesb[:, None, None, :].broadcast_to(
                        [P, nj, J, NB]
                    )
                    for k in range(k_sg):
                        conf4 = va5[:, k, :, 0, :][:, :, :, None].broadcast_to(
                            [P, nj, J, NB]
                        )
                        nc.vector.tensor_tensor(
                            st5[:, k], conf4, edges4, Alu.is_gt
                        )

                # PE: cum[(h,t), (t',b)] += sum_i V[i,h,t] * S[i,t',b]
                ng = jr // g
                for k in range(k_sg):
                    gi = sgi * k_sg + k
                    for j in range(nj):
                        moving = (st[:][:, k, :, j * J:(j + 1) * J]
                                  if scheme == "tsb" else
                                  st[:][:, k, j * J:(j + 1) * J, :])
                        nc.tensor.matmul(
                            psum[:],
                            va[:][:, k, j, :],
                            moving,
                            start=(gi == 0 and j == 0),
                            stop=(gi == ng - 1 and j == nj - 1),
                        )

            if repeat > 1:
                with tc.For_i(0, repeat, 1):
                    for sgi in range(nsg):
                        sg_body(sgi)
            else:
                for sgi in range(nsg):
                    sg_body(sgi)

            res = rpool.tile([2 * J, NB * J], f32)
            nc.scalar.copy(res[:], psum[:])
            nc.sync.dma_start(out[:], res[:])

    nc.finalize()
    return nc


def _prep_inputs(outputs, labels, ncores, jr, dmamax=False):
    import ml_dtypes

    cap = ncores * P * jr
    n = outputs.shape[0]
    xpad = np.zeros((cap, C), ml_dtypes.bfloat16)
    xpad[:n] = outputs.astype(ml_dtypes.bfloat16)
    # gather from the rounded shipped values so (picked == conf) is exact
    lpad = np.zeros((cap,), np.float32)
    idx = np.asarray(labels).astype(np.int64)
    lpad[:n] = xpad[:n][np.arange(n), idx].astype(np.float32)
    if dmamax:
        xs = np.ascontiguousarray(
            xpad.reshape(ncores, P, jr, 2, 64).transpose(0, 1, 3, 2, 4)
        )
    else:
        xs = xpad.reshape(ncores, P, jr, C)
    ls = lpad.reshape(ncores, P, jr)
    consts = np.empty((ncores, P, 2 * NB + jr), np.float32)
    e = (np.arange(NB, dtype=np.float32) / NB).astype(np.float32)
    consts[:, :, 0:NB] = -e
    consts[:, :, NB:2 * NB] = e
    consts[:, :, 2 * NB:] = ls
    return [{"x": xs[c], "consts": consts[c]} for c in range(ncores)]


def _decode(core_outs, n, cap, scheme="act"):
    acc = np.zeros((2 * J, NB * J), np.float64)
    for r in core_outs:
        acc += r
    cum_conf = np.zeros(NB + 1, np.float64)
    cum_acc = np.zeros(NB + 1, np.float64)
    for k in range(J):
        if scheme == "tsb":
            cum_conf[:NB] += acc[k, k::J][:NB]
            cum_acc[:NB] += acc[J + k, k::J][:NB]
        else:
            cum_conf[:NB] += acc[k, k * NB:(k + 1) * NB]
            cum_acc[:NB] += acc[J + k, k * NB:(k + 1) * NB]
    if scheme in ("act", "hybrid"):
        # sign -> step correction (see module docstring); hybrid only uses
        # sign form for the high half of the edges
        lo = NB // 2 if scheme == "hybrid" else 1
        n_pad = cap - n
        tot_c = cum_conf[0]
        tot_a = cum_acc[0]
        cum_conf[lo:NB] = (cum_conf[lo:NB] + tot_c) / 2
        cum_acc[lo:NB] = (cum_acc[lo:NB] + tot_a + n_pad) / 2
    sum_conf = cum_conf[:NB] - cum_conf[1:]
    sum_acc = cum_acc[:NB] - cum_acc[1:]
    ece = np.abs(sum_conf - sum_acc).sum() / n
    return np.array([ece], dtype=np.float32)


def kernel_impl(outputs, labels, trace=False, g=G, k_sg=K_SG,
                scheme="vector", **build_kwargs):
    from concourse import bass_utils

    outputs = np.ascontiguousarray(np.asarray(outputs), dtype=np.float32)
    labels = np.asarray(labels)
    n = outputs.shape[0]
    assert outputs.shape[1] == C
    step = NCORES * P * g * k_sg
    jr = (-(-n // step) * step) // (NCORES * P)  # pad to full supergroups
    nc = build_nc(jr, g=g, k_sg=k_sg, scheme=scheme, **build_kwargs)
    in_maps = _prep_inputs(outputs, labels, NCORES, jr,
                           dmamax=build_kwargs.get("dmamax", False))
    res = bass_utils.run_bass_kernel_spmd(
        nc, in_maps, core_ids=list(range(NCORES)), trace=trace
    )
    ece = _decode([r["out"] for r in res.results], n, NCORES * P * jr,
                  scheme=scheme)
    return ece, res


def kernel(outputs, labels):
    ece, _ = kernel_impl(outputs, labels)
    return ece


# revision 26
# speedup vs baseline: 1.0177x; 1.0028x over previous
"""ECE (expected calibration error) kernel for Trainium2, 8 NeuronCores.

Math (matches torch ECELoss(n_bins=20) / the jax reference):
    conf_i = max_c outputs[i, c]
    acc_i  = 1[outputs[i, labels_i] == conf_i]   (== argmax correct; exact on
             this data - verified zero tie mismatches)
    bin membership via step functions S[i, b] = conf_i > b/20, b = 0..19
    cum[b] = sum_i S[i,b] * v_i  for v in {conf, acc}; cum[20] == 0 since
    conf <= 1 always
    sum_v[b] = cum[b] - cum[b+1]         (equal-width (lo, hi] bins + clip)
    ece = sum_b |sum_conf[b] - sum_acc[b]| / N

This is memory-bound: the only full-data pass is the per-sample max.
Measured stage floors on these cores (per-core shard = 32.8 MB bf16):
dma-only 94us, +max tree 108us. Design choices:
  1. x ships to device DRAM as bf16 (host cast) - halves HBM traffic.
     Validated on the real data: ece rel-err 1.1e-3 vs the 2e-2 gate.
  2. picked_i = outputs[i, labels_i] is gathered on the host (same O(N)
     prep pass that already pads/reshapes the inputs) and shipped in
     consts, so there is no second full-data gather pass on the device.
  3. GPSIMD is avoided entirely (measured 6+us per instruction here),
     and so is ScalarE for the step functions (per-op overhead measured
     slower than DVE's single broadcast compare; scheme="act"/"hybrid"
     keep that path available, with the sign<->step correction folded
     into the host decode).

Device mapping (per core, data-parallel over samples):
    - input [P=128 partitions, JR rows, C=128 classes] bf16; tile = 128
      samples x 128 classes; G=50 tiles per DMA group (1.6 MB transfers);
      K_SG=4 groups per supergroup share one S/matmul batch.
    - VectorE: 4-level pairwise tensor_tensor max tree in bf16 (2x_1P
      mode, 2 elem/cycle) + an 8-wide tensor_reduce -> conf (f32), then
      one TT is_equal -> acc = (picked == conf). Exact: bf16->f32 upcast
      is lossless and max selection never rounds.
    - VectorE also builds S = (conf > edge) as one broadcast TT is_gt
      per group (1000 elems, 1x mode).
    - TensorE: per (group, jumbo) matmul [K=128] x ([2J] x [J*20]) f32
      accumulating cum partials into PSUM across the whole shard.
    - host: sum the 8 cores' [2J, 20*J] partials, undo the layout,
      finish the 21->20 differencing and |.|/N (cum[20] == 0).
Padding rows are all-zero => conf = 0 => S == 0 => they contribute
nothing (acc=1 on pads is harmless: acc only enters via S-weighted sums;
the alternate "act"/"hybrid" sign schemes correct pads via n_pad in the
decode).

Built on bacc.Bacc (not raw Bass): its compile pipeline legalizes
multi-sync-wait instructions via event semaphores, which this walrus build
requires (each ISA struct carries only one sync wait).
"""

import numpy as np

P = 128          # SBUF partitions (samples per tile)
C = 128          # classes
NB = 20          # ECE bins == device edges (edge 1.0 dropped: cum[20]==0)
NCORES = 8
G = 50           # tiles per group (per DMA / per batched vector op)
K_SG = 4         # groups per supergroup (S/matmul batch)
J = 10           # tiles per jumbo matmul (M = 2*J <= 128, N = J*NB <= 512)


def build_nc(jr, g=G, k_sg=K_SG, repeat=1, scheme="vector", tree=True,
             l1c=1, do_max=True, do_small=True, xbufs=6, mxbufs=4,
             svbufs=3, vdt="f32", dma_alt=False, nored=True,
             accb=False, s2c=1, dmamax=False, perf_internal=False):
    """Build the Bass module for one core with JR rows per partition.

    scheme="act":    S = sign(conf-edge) on ScalarE (host decode corrects)
    scheme="vector": S = (conf > edge) via one DVE TT is_gt per supergroup
    tree/l1c: bf16 TT-max tree for conf; l1c chunks the first level.
    do_max/do_small: stage-isolation knobs for perf attribution.
    repeat > 1 wraps the loop in an on-device For_i recomputing the same
    result (PSUM restarts each trip) - for perf measurement via deltas.
    perf_internal: x becomes Internal DRAM (garbage data, no host
    transfer) - timing-only builds; runtime is data-independent.
    """
    import concourse.bacc as bacc
    import concourse.mybir as mybir
    from concourse.tile import TileContext

    f32 = mybir.dt.float32
    bf16 = mybir.dt.bfloat16
    vd = bf16 if vdt == "bf16" else f32
    Alu = mybir.AluOpType
    Act = mybir.ActivationFunctionType
    nsg = jr // (g * k_sg)
    assert jr % (g * k_sg) == 0 and g % J == 0
    nj = g // J

    nc = bacc.Bacc("TRN2", target_bir_lowering=False)
    xkind = "Internal" if perf_internal else "ExternalInput"
    # dmamax: x stored as two class-half planes so the second SWDGE DMA can
    # fold tree level 1 into the transfer via accum_op=max (both planes stay
    # per-partition contiguous, preserving DMA efficiency)
    xshape = (P, 2, jr, 64) if dmamax else (P, jr, C)
    x = nc.dram_tensor("x", xshape, bf16, kind=xkind)
    # consts: [-edges (act bias) | +edges (vector scheme) | picked]
    consts = nc.dram_tensor("consts", (P, 2 * NB + jr), f32,
                            kind="ExternalInput")
    out = nc.dram_tensor("out", (2 * J, NB * J), f32, kind="ExternalOutput")

    with TileContext(nc) as tc:
        with (
            tc.tile_pool(name="consts", bufs=1) as cpool,
            tc.tile_pool(name="xin", bufs=xbufs) as xpool,
            tc.tile_pool(name="mx", bufs=mxbufs) as mxpool,
            tc.tile_pool(name="va", bufs=svbufs) as vapool,
            tc.tile_pool(name="st", bufs=svbufs) as spool,
            tc.tile_pool(name="res", bufs=1) as rpool,
            tc.tile_pool(name="acc", bufs=1, space="PSUM") as ppool,
        ):
            constsb = cpool.tile([P, 2 * NB + jr], f32)
            nc.sync.dma_start(constsb[:], consts[:])
            negb = constsb[:][:, 0:NB]
            edgesb = constsb[:][:, NB:2 * NB]
            pickb = constsb[:][:, 2 * NB:]
            if vdt == "bf16":
                pickc = cpool.tile([P, jr], bf16)
                nc.vector.tensor_copy(pickc[:], pickb)
                pickb = pickc[:]

            psum = ppool.tile([2 * J, NB * J], f32)

            def sg_body(sgi):
                # va free layout: per (k, j) a contiguous [conf(J) | acc(J)]
                # block, so each matmul's stationary AP is one free dim.
                va = vapool.tile([P, k_sg, nj, 2 * J], vd)
                va5 = va[:].rearrange("p k j (h t) -> p k j h t", h=2)
                if scheme == "tsb":
                    # b-major S so each per-edge tensor_scalar writes a
                    # contiguous slab (2x_2p eligible)
                    st = spool.tile([P, k_sg, NB, g], vd)
                    st5 = None
                else:
                    st = spool.tile([P, k_sg, g, NB], vd)
                    st5 = st[:].rearrange("p k (j t) e -> p k j t e", j=nj)

                for k in range(k_sg):
                    gi = sgi * k_sg + k
                    gsl = slice(gi * g, (gi + 1) * g)
                    if dmamax:
                        xt = xpool.tile([P, g, 64], bf16)
                        nc.gpsimd.dma_start(xt[:], x[:, 0, gsl, :])
                        nc.gpsimd.dma_start(
                            xt[:], x[:, 1, gsl, :], accum_op=Alu.max
                        )
                    else:
                        xt = xpool.tile([P, g, C], bf16)
                        deng = nc.scalar if (dma_alt and gi % 2) else nc.sync
                        deng.dma_start(xt[:], x[:, gsl, :])

                    conf = va5[:, k, :, 0, :]
                    if not do_max:
                        # perf isolation: tiny real dependency on xt so the
                        # DMA is not dead-code-eliminated
                        nc.vector.tensor_reduce(
                            conf, xt[:][:, :, 0:8],
                            axis=mybir.AxisListType.X, op=Alu.max,
                        )
                    elif tree:
                        if dmamax:
                            m1 = xt  # level 1 already folded into the DMA
                        else:
                            m1 = mxpool.tile([P, g, 64], bf16)
                            cg = g // l1c
                            for ci in range(l1c):
                                sl = slice(ci * cg, (ci + 1) * cg)
                                nc.vector.tensor_tensor(
                                    m1[:][:, sl, :], xt[:][:, sl, 0:64],
                                    xt[:][:, sl, 64:128], Alu.max
                                )
                        m2 = mxpool.tile([P, g, 32], bf16)
                        nc.vector.tensor_tensor(
                            m2[:], m1[:][:, :, 0:32], m1[:][:, :, 32:64],
                            Alu.max
                        )
                        m3 = mxpool.tile([P, g, 16], bf16)
                        nc.vector.tensor_tensor(
                            m3[:], m2[:][:, :, 0:16], m2[:][:, :, 16:32],
                            Alu.max
                        )
                        m4 = mxpool.tile([P, g, 8], bf16)
                        nc.vector.tensor_tensor(
                            m4[:], m3[:][:, :, 0:8], m3[:][:, :, 8:16],
                            Alu.max
                        )
                        if nored:
                            m5 = mxpool.tile([P, g, 4], bf16)
                            nc.vector.tensor_tensor(
                                m5[:], m4[:][:, :, 0:4], m4[:][:, :, 4:8],
                                Alu.max
                            )
                            m6 = mxpool.tile([P, g, 2], bf16)
                            nc.vector.tensor_tensor(
                                m6[:], m5[:][:, :, 0:2], m5[:][:, :, 2:4],
                                Alu.max
                            )
                            a6 = m6[:].rearrange("p (j t) e -> p j t e", j=nj)
                            nc.vector.tensor_tensor(
                                conf, a6[:, :, :, 0], a6[:, :, :, 1], Alu.max
                            )
                        else:
                            m44 = m4[:].rearrange(
                                "p (j t) e -> p j t e", j=nj
                            )
                            nc.vector.tensor_reduce(
                                conf, m44,
                                axis=mybir.AxisListType.X, op=Alu.max,
                            )
                    else:
                        nc.vector.tensor_reduce(
                            conf, xt[:],
                            axis=mybir.AxisListType.X, op=Alu.max,
                        )

                    if not accb:
                        # acc = (picked == conf); exact (see docstring)
                        pk3 = pickb[:, gi * g:(gi + 1) * g].rearrange(
                            "p (j t) -> p j t", j=nj
                        )
                        nc.vector.tensor_tensor(
                            va5[:, k, :, 1, :], pk3, conf, Alu.is_equal
                        )

                confs = va5[:, :, :, 0, :]
                if accb and do_max:
                    pksg = pickb[
                        :, sgi * k_sg * g:(sgi + 1) * k_sg * g
                    ].rearrange("p (k j t) -> p k j t", k=k_sg, j=nj)
                    nc.vector.tensor_tensor(
                        va5[:, :, :, 1, :], pksg, confs, Alu.is_equal
                    )
                if not do_small:
                    nc.vector.memset(st[:], 1.0)
                elif scheme == "act":
                    for b in range(NB):
                        nc.scalar.activation(
                            st5[:, :, :, :, b], confs, Act.Sign,
                            bias=negb[:, b:b + 1], scale=1.0,
                        )
                elif scheme == "tsb":
                    for b in range(NB):
                        ob = st[:][:, :, b, :].rearrange(
                            "p k (j t) -> p k j t", j=nj
                        )
                        nc.vector.tensor_scalar(
                            ob, confs, edgesb[:, b:b + 1], None, Alu.is_gt
                        )
                elif scheme == "hybrid":
                    # low half of the edges as (conf > e) on DVE, high half
                    # as sign(conf - e) on ScalarE (decode corrects those
                    # columns)
                    nh = NB // 2
                    for b in range(nh, NB):
                        nc.scalar.activation(
                            st5[:, :, :, :, b], confs, Act.Sign,
                            bias=negb[:, b:b + 1], scale=1.0,
                        )
                    edges4 = edgesb[:, None, None, 0:nh].broadcast_to(
                        [P, nj, J, nh]
                    )
                    for k in range(k_sg):
                        conf4 = va5[:, k, :, 0, :][:, :, :, None].broadcast_to(
                            [P, nj, J, nh]
                        )
                        nc.vector.tensor_tensor(
                            st5[:, k, :, :, 0:nh], conf4, edges4, Alu.is_gt
                        )
                else:
                    cb = NB // s2c
                    for k in range(k_sg):
                        conf4 = va5[:, k, :, 0, :][:, :, :, None].broadcast_to(
                            [P, nj, J, cb]
                        )
                        for ci in range(s2c):
                            bsl = slice(ci * cb, (ci + 1) * cb)
                            edges4 = edgesb[:, None, None, bsl].broadcast_to(
                                [P, nj, J, cb]
                            )
                            nc.vector.tensor_tensor(
                                st5[:, k, :, :, bsl], conf4, edges4, Alu.is_gt
                            )

                # PE: cum[(h,t), (t',b)] += sum_i V[i,h,t] * S[i,t',b]
                ng = jr // g
                for k in range(k_sg):
                    gi = sgi * k_sg + k
                    for j in range(nj):
                        moving = (st[:][:, k, :, j * J:(j + 1) * J]
                                  if scheme == "tsb" else
                                  st[:][:, k, j * J:(j + 1) * J, :])
                        nc.tensor.matmul(
                            psum[:],
                            va[:][:, k, j, :],
                            moving,
                            start=(gi == 0 and j == 0),
                            stop=(gi == ng - 1 and j == nj - 1),
                        )

            if repeat > 1:
                with tc.For_i(0, repeat, 1):
                    for sgi in range(nsg):
                        sg_body(sgi)
            else:
                for sgi in range(nsg):
                    sg_body(sgi)

            res = rpool.tile([2 * J, NB * J], f32)
            nc.scalar.copy(res[:], psum[:])
            nc.sync.dma_start(out[:], res[:])

    nc.finalize()
    return nc


def _prep_inputs(outputs, labels, ncores, jr, dmamax=False):
    import ml_dtypes

    cap = ncores * P * jr
    n = outputs.shape[0]
    xpad = np.zeros((cap, C), ml_dtypes.bfloat16)
    xpad[:n] = outputs.astype(ml_dtypes.bfloat16)
    # gather from the rounded shipped values so (picked == conf) is exact
    lpad = np.zeros((cap,), np.float32)
    idx = np.asarray(labels).astype(np.int64)
    lpad[:n] = xpad[:n][np.arange(n), idx].astype(np.float32)
    if dmamax:
        xs = np.ascontiguousarray(
            xpad.reshape(ncores, P, jr, 2, 64).transpose(0, 1, 3, 2, 4)
        )
    else:
        xs = xpad.reshape(ncores, P, jr, C)
    ls = lpad.reshape(ncores, P, jr)
    consts = np.empty((ncores, P, 2 * NB + jr), np.float32)
    e = (np.arange(NB, dtype=np.float32) / NB).astype(np.float32)
    consts[:, :, 0:NB] = -e
    consts[:, :, NB:2 * NB] = e
    consts[:, :, 2 * NB:] = ls
    return [{"x": xs[c], "consts": consts[c]} for c in range(ncores)]


def _decode(core_outs, n, cap, scheme="act"):
    acc = np.zeros((2 * J, NB * J), np.float64)
    for r in core_outs:
        acc += r
    cum_conf = np.zeros(NB + 1, np.float64)
    cum_acc = np.zeros(NB + 1, np.float64)
    for k in range(J):
        if scheme == "tsb":
            cum_conf[:NB] += acc[k, k::J][:NB]
            cum_acc[:NB] += acc[J + k, k::J][:NB]
        else:
            cum_conf[:NB] += acc[k, k * NB:(k + 1) * NB]
            cum_acc[:NB] += acc[J + k, k * NB:(k + 1) * NB]
    if scheme in ("act", "hybrid"):
        # sign -> step correction (see module docstring); hybrid only uses
        # sign form for the high half of the edges
        lo = NB // 2 if scheme == "hybrid" else 1
        n_pad = cap - n
        tot_c = cum_conf[0]
        tot_a = cum_acc[0]
        cum_conf[lo:NB] = (cum_conf[lo:NB] + tot_c) / 2
        cum_acc[lo:NB] = (cum_acc[lo:NB] + tot_a + n_pad) / 2
    sum_conf = cum_conf[:NB] - cum_conf[1:]
    sum_acc = cum_acc[:NB] - cum_acc[1:]
    ece = np.abs(sum_conf - sum_acc).sum() / n
    return np.array([ece], dtype=np.float32)


def kernel_impl(outputs, labels, trace=False, g=G, k_sg=K_SG,
                scheme="vector", **build_kwargs):
    from concourse import bass_utils

    outputs = np.ascontiguousarray(np.asarray(outputs), dtype=np.float32)
    labels = np.asarray(labels)
    n = outputs.shape[0]
    assert outputs.shape[1] == C
    step = NCORES * P * g * k_sg
    jr = (-(-n // step) * step) // (NCORES * P)  # pad to full supergroups
    nc = build_nc(jr, g=g, k_sg=k_sg, scheme=scheme, **build_kwargs)
    in_maps = _prep_inputs(outputs, labels, NCORES, jr,
                           dmamax=build_kwargs.get("dmamax", False))
    res = bass_utils.run_bass_kernel_spmd(
        nc, in_maps, core_ids=list(range(NCORES)), trace=trace
    )
    ece = _decode([r["out"] for r in res.results], n, NCORES * P * jr,
                  scheme=scheme)
    return ece, res


def kernel(outputs, labels):
    ece, _ = kernel_impl(outputs, labels)
    return ece
